# revision 57
# baseline (speedup 1.0000x reference)
"""Trainium2 Bass kernel for the Backflow module.

Math (B=16, N=512, DIM=3, H=32):
  out[b,i,:] = sum_j eta(||x_bi - x_bj||) * (x_bi - x_bj)  +  mu(||x_bi||) * x_bi
where eta/mu are 1->H->1 tanh MLPs. The reference's eye()/diagonal correction
cancels exactly (eta(0)*(x_i - x_i) = 0 in the matrix form below).

Sharding: data-parallel over batch, 2 batches per core on 8 cores.

eta and mu are univariate scalar functions and the rel-err budget (2e-2)
is large, so we fit cheap surrogates at call time from the actual weights,
both in u = d^2 (no sqrt anywhere; exp/identity/copy live in one ACT
table set -> a single table load):

  t[i,j] = 2*d_ij^2/umax - 1 comes straight out of the PE: the d^2
  matmul carries two extra rows ([-2sx | s | s*n2_i - 1] stationary x
  [x | n2_j | 1] moving, fp16 = 1 cyc/row) so PSUM holds t directly;
  ACT copies the A-columns and DVE the B-columns to SBUF.

  M[i,j] := -eta(d_ij) - c0 evaluated two ways on disjoint column regions
  of the packed strip:
   A-region (ACT+PE): sum_m c_m exp(g_m (t+1)) - META exp ACT passes,
     |c_m| folded into the bias, sign via +/-identity fp16 stationaries
     accumulated on the PE into PSUM; one ACT copy -> bf16 M tile.
   B-region (DVE): monomial Horner for P(t) - c0 via stock
     scalar_tensor_tensor ops (g = c_deg*t; g = (g + c_j)*t).
  The split ratio load-balances ACT vs DVE.

  The shared constant c0 is folded into the finalize for free:
  out_c[j] = (P'_c[j] + c0*X_c) - x_c[j]*(Q'[j] + c0*N), X_c = sum_i x_c[i].

  mu(||x_i||) = c0' + sum_m c'_m exp(-b_m n_i^2): ONE ACT exp pass on a
  [MU, N] broadcast of n^2 (per-partition scale), folded into the Q rows
  of the PSUM contraction with a negated bf16 stationary.

Per-core layout: i on partitions (4 chunks of 128), j on the free dim.
Symmetry eta(d_ij) = eta(d_ji): compute only block-triangular strips
(chunk I covers j in [128*I, 512)), packed to [128, 1280] with
bank-aligned chunk offsets (order 0,1,3,2) so every matmul output stays
inside a PSUM bank.

Row sums via PE contractions (3-wide ones / x stationaries in bf16, M
moving in bf16 = 1 cyc/row), merged per stationary chunk: the direct
contributions of stationary chunk I cover the contiguous strip
[OFFS[I], OFFS[I]+W) -> one matmul per (I, P/Q); the reflected blocks
(via 6 PE transposes into ONE PSUM bank, one DVE copy back, ordered by
J) also merge per stationary J. Interleaving b1's d^2 matmuls into
b0's accumulate gaps keeps the PE p-state up. Input DMAs are packed
into few tensors and triggered from different engines' queues.
"""

import sys

sys.path.insert(0, "/opt/trn_rl_repo")

import numpy as np
from contextlib import ExitStack

B, N, DIM, H = 16, 512, 3, 32
NCORES = 8
BPC = B // NCORES  # batches per core
P = 128
NCHUNK = N // P  # 4
WIDTHS = [N - P * I for I in range(NCHUNK)]  # [512, 384, 256, 128]
# bank-aligned packing of the block-triangular strips (chunk order 0,1,3,2)
OFFS = [0, 512, 1024, 896]
NPACK = sum(WIDTHS)  # 1280

DEG = 8  # B-region polynomial degree
META = 5  # A-region exp basis size (batch 0)
META1 = 5  # tail-batch exp basis size (5 = symmetric; 4 trades error for chain)
MU = 12  # mu exp-basis size (incl. the g=0 constant term)
ASPLIT = 832  # packed columns [0, ASPLIT) on ACT path, rest on DVE path
ASEGS = [(0, 512), (512, 320)]  # accumulate matmul splits (PSUM banks, >=256)
assert ASEGS[-1][0] + ASEGS[-1][1] == ASPLIT

# transposed-block pairs ordered by stationary chunk J, then I;
# J-group g starts at index J*(J-1)/2 and holds J blocks (I = 0..J-1)
PAIRS_BYJ = [(I, J) for J in range(1, NCHUNK) for I in range(J)]

LAST_RESULT = None


def _spread_sync_waits(nc):
    """The pinned walrus rejects instructions carrying more than one sync wait
    ('Too many sync wait commands'). Engines execute their instruction streams
    in order, so hoist all-but-one wait of any such instruction onto same-engine
    NoOps inserted directly before it — semantically identical ordering."""
    from concourse import mybir

    n_added = 0
    for bb in nc.main_func.blocks:
        insts = bb.instructions
        i = 0
        while i < len(insts):
            inst = insts[i]
            si = getattr(inst, "sync_info", None)
            waits = list(si.on_wait) if si is not None and si.on_wait else []
            if len(waits) > 1:
                si.on_wait = waits[-1:]
                for k, w in enumerate(waits[:-1]):
                    nop = mybir.InstNoOp(
                        name=f"{inst.name}-wspread{k}",
                        sync_info=mybir.SyncInfo(on_wait=[w], on_update=[]),
                        engine=inst.engine,
                        bass_nofuse=True,
                    )
                    insts.insert(i + k, nop)
                    n_added += 1
                i += len(waits) - 1
            i += 1
    return n_added


def _eta_fn(d, w1, b1, w2, b2):
    return np.tanh(d[..., None] * w1[0] + b1) @ w2[:, 0] + b2[0]


def _fit_surrogates(x, eta_w1, eta_b1, eta_w2, eta_b2):
    """Global fits of f(t) = -eta(sqrt(u)), t = 2u/umax - 1:
    poly (ascending monomial coeffs, deg DEG) and exp basis
    f - c0 ~= sum_m c_m exp(g_m (t+1)). Returns (s, pc, gam, ce)."""
    x = x.astype(np.float64)
    n2 = (x**2).sum(-1)  # [B, N]
    rng = np.random.default_rng(0)
    umax = 0.0
    samples = []
    for b in range(B):
        G = x[b] @ x[b].T
        Ub = np.maximum(n2[b][:, None] + n2[b][None, :] - 2 * G, 0.0)
        umax = max(umax, float(Ub.max()))
        idx = rng.choice(N * N, 16384, replace=False)
        samples.append(Ub.reshape(-1)[idx])
    umax = umax * 1.002 + 1e-6
    uu = np.concatenate(samples)
    ug = np.linspace(0.0, umax, 2000)
    ufit = np.concatenate([uu, ug])
    w = np.concatenate(
        [np.sqrt(np.sqrt(uu) + 0.1), 3.0 * np.sqrt(np.sqrt(ug) + 0.1)]
    )
    tfit = 2.0 * ufit / umax - 1.0
    yfit = -_eta_fn(np.sqrt(ufit), eta_w1, eta_b1, eta_w2, eta_b2)
    import numpy.polynomial.chebyshev as Ch

    cf = Ch.chebfit(tfit, yfit, DEG, w=w)
    pc = Ch.cheb2poly(cf)  # ascending monomial coeffs in t
    c0 = float(pc[0])
    # exp basis on the residual target f - c0, no free constant;
    # a second, smaller basis for the tail batch (shorter serial chain)
    def expfit(m):
        gam = -np.geomspace(0.08, 48.0, m)
        A = np.exp((tfit[:, None] + 1.0) * gam[None, :])
        ce, *_ = np.linalg.lstsq(A * w[:, None], (yfit - c0) * w, rcond=None)
        return gam, ce

    gam, ce = expfit(META)
    gam4, ce4 = expfit(META1)
    s = 2.0 / umax
    return float(s), pc.astype(np.float64), (gam, ce), (gam4, ce4)


def _fit_mu_exp(n2_all, mu_w1, mu_b1, mu_w2, mu_b2):
    """Fit mu(sqrt(u)) ~= sum_m c_m exp(-g_m u) on the actual n^2 values
    (the exact evaluation points). g_0 = 0 supplies the constant term."""
    us = np.sort(n2_all.reshape(-1).astype(np.float64))
    n2max = float(us[-1]) * 1.001 + 1e-9
    g = np.concatenate([[0.0], np.geomspace(0.125, 96.0, MU - 1) / n2max])
    A = np.exp(-us[:, None] * g[None, :])
    y = _eta_fn(np.sqrt(us), mu_w1, mu_b1, mu_w2, mu_b2)
    w = np.sqrt(np.sqrt(us) + 0.1)
    Aw = A * w[:, None]
    AtA = Aw.T @ Aw + 1e-10 * len(us) * np.eye(MU)
    c = np.linalg.solve(AtA, Aw.T @ (y * w))
    return g.astype(np.float64), c.astype(np.float64)


# packed f32 smalls blob layout: [P, FPK] with
#   cols [0, META+1): eab (exp-basis biases + mu zero bias col)
#   col META+1: negbeta (rows 0:MU)
#   cols META+2 .. META+3: c0x (rows 0:DIM)
FPK = META + META1 + 2 + BPC


def _build_program(poly_pc, eta_e0, eta_e1):
    import concourse.bass as bass
    import concourse.tile as tile
    from concourse import mybir

    f32 = mybir.dt.float32
    f16 = mybir.dt.float16
    bf16 = mybir.dt.bfloat16
    AF = mybir.ActivationFunctionType
    OP = mybir.AluOpType

    pc = [float(v) for v in poly_pc]  # ascending, len DEG+1
    c0 = pc[0]
    ea_scale = [[float(g) for g in e[0]] for e in (eta_e0, eta_e1)]
    ea_sign = [[1.0 if c > 0 else -1.0 for c in e[1]] for e in (eta_e0, eta_e1)]
    NM = [META, META1]
    BCOL = [0, META]  # eab bias column offset per batch

    DR = DIM + 2  # d^2 matmul rows: x(3), n2, ones
    NPAIR = len(PAIRS_BYJ)

    nc = bass.Bass()
    # dfirst: statd(b0,I0) | xTn(b0)  (fp16, gates the first matmul)
    dfirst_d = nc.dram_tensor("dfirst", [DR, P + N], f16, kind="ExternalInput")
    # dpack: statd for the other 7 (b,I) chunks | xTn(b1)
    dpack_d = nc.dram_tensor("dpack", [DR, 7 * P + N], f16, kind="ExternalInput")
    identh_d = nc.dram_tensor("identh", [P, 2, P], f16, kind="ExternalInput")
    # bpack: statx6 (BPC*NCHUNK*6 cols) | identb (P cols), rows 0:MU of the
    # first 3 cols after that hold muAb
    BPK = BPC * NCHUNK * 2 * DIM + P + DIM
    bpack_d = nc.dram_tensor("bpack", [P, BPK], bf16, kind="ExternalInput")
    xb_d = nc.dram_tensor("xb", [DIM, BPC, N], bf16, kind="ExternalInput")
    unrep_d = nc.dram_tensor("unrep", [MU, BPC, N], f16, kind="ExternalInput")
    fpack_d = nc.dram_tensor("fpack", [P, FPK], f32, kind="ExternalInput")
    out_d = nc.dram_tensor("out", [BPC, DIM, N], f32, kind="ExternalOutput")

    with tile.TileContext(nc) as tc, ExitStack() as ctx:
        singles = ctx.enter_context(tc.tile_pool(name="singles", bufs=1))
        tap = ctx.enter_context(tc.tile_pool(name="tap", bufs=2))
        tbp = ctx.enter_context(tc.tile_pool(name="tbp", bufs=2))
        hpool = ctx.enter_context(tc.tile_pool(name="hpool", bufs=2))
        hsp = ctx.enter_context(tc.tile_pool(name="hsp", bufs=8))
        mpool = ctx.enter_context(tc.tile_pool(name="mpool", bufs=2))
        atp = ctx.enter_context(tc.tile_pool(name="atp", bufs=2))
        hmup = ctx.enter_context(tc.tile_pool(name="hmup", bufs=2))
        finp = ctx.enter_context(tc.tile_pool(name="finp", bufs=2))
        orp = ctx.enter_context(tc.tile_pool(name="orp", bufs=2))
        psd2 = ctx.enter_context(tc.tile_pool(name="psd2", bufs=1, space="PSUM"))
        psacc = ctx.enter_context(tc.tile_pool(name="psacc", bufs=1, space="PSUM"))
        psout = ctx.enter_context(tc.tile_pool(name="psout", bufs=1, space="PSUM"))
        pstr = ctx.enter_context(tc.tile_pool(name="pstr", bufs=1, space="PSUM"))

        # ---- inputs, spread across engine DMA queues; critical pack first --
        dfirst_sb = singles.tile([DR, P + N], f16)
        nc.scalar.dma_start(out=dfirst_sb[:], in_=dfirst_d[:])
        dpack_sb = singles.tile([DR, 7 * P + N], f16)
        nc.scalar.dma_start(out=dpack_sb[:], in_=dpack_d[:])
        fpack_sb = singles.tile([P, FPK], f32)
        nc.sync.dma_start(out=fpack_sb[:], in_=fpack_d[:])
        identh = singles.tile([P, 2, P], f16)  # [:,0,:]=+I, [:,1,:]=-I
        nc.scalar.dma_start(out=identh[:], in_=identh_d[:])
        bpack_sb = singles.tile([P, BPK], bf16)
        nc.scalar.dma_start(out=bpack_sb[:], in_=bpack_d[:])
        xb_sb = singles.tile([DIM, BPC, N], bf16)
        nc.sync.dma_start(out=xb_sb[:], in_=xb_d[:])
        unrep_sb = singles.tile([MU, BPC, N], f16)
        nc.sync.dma_start(out=unrep_sb[:], in_=unrep_d[:])

        # views into the packs
        SD = 7 * P  # statd column count in dpack (all chunks except b0,I0)

        def statd_v(b, I):
            if b == 0 and I == 0:
                return dfirst_sb[:, 0:P]
            off = (b * NCHUNK + I - 1) * P
            return dpack_sb[:, off : off + P]

        def xTn_v(b, j0, j1):
            if b == 0:
                return dfirst_sb[:, P + j0 : P + j1]
            return dpack_sb[:, SD + j0 : SD + j1]

        def statx6_v(b, I, c0_, c1_):
            off = (b * NCHUNK + I) * 2 * DIM
            return bpack_sb[:, off + c0_ : off + c1_]

        identb = bpack_sb[:, BPC * NCHUNK * 2 * DIM : BPC * NCHUNK * 2 * DIM + P]
        muAb = bpack_sb[0:MU, BPK - DIM : BPK]
        eab = fpack_sb[:, 0 : META + META1 + 1]
        MUB = META + META1  # mu zero-bias col
        negbeta = fpack_sb[0:MU, MUB + 1 : MUB + 2]
        c0x = fpack_sb[0:DIM, MUB + 2 : MUB + 2 + BPC]

        # one exp pass covers both batches' mu hidden layer
        hmu_all = hmup.tile([MU, BPC, N], bf16, tag="hmu")
        nc.scalar.activation(
            hmu_all[:],
            unrep_sb[:],
            AF.Exp,
            scale=negbeta[:, 0:1],
            bias=eab[0:MU, MUB : MUB + 1],
        )

        # ---- t strips straight from the PE (fp16 operands, 1 cyc/row) ----
        def emit_d2_chunk(b, I, tps):
            nc.tensor.matmul(
                tps[:, OFFS[I] : OFFS[I] + WIDTHS[I]],
                statd_v(b, I),
                xTn_v(b, P * I, N),
                start=True,
                stop=True,
                skip_group_check=True,
            )

        BW = NPACK - ASPLIT  # B-region width per batch

        def emit_tA(b, tps):
            tA = tap.tile([P, ASPLIT], f32, tag="ta")
            nc.vector.tensor_copy(tA[:], tps[:, 0:ASPLIT])
            return tA

        def emit_tB(b, tps, tBall):
            nc.vector.tensor_copy(
                tBall[:, b * BW : (b + 1) * BW], tps[:, ASPLIT:NPACK]
            )

        def emit_expacc(b, t_ap, interleave=None):
            """A-region: exp passes on ACT, +/-I fp16 accumulate on PE.
            `interleave` emits one extra PE op after each m (p-state filler)."""
            nm = NM[b]
            acc = psacc.tile([P, ASPLIT], f32, tag="acc")
            for m in range(nm):
                hs = hsp.tile([P, ASPLIT], f16, tag="hs")
                nc.scalar.activation(
                    hs[:],
                    t_ap,
                    AF.Exp,
                    scale=ea_scale[b][m],
                    bias=eab[:, BCOL[b] + m : BCOL[b] + m + 1],
                )
                sgn = 0 if ea_sign[b][m] > 0 else 1
                for off, w in ASEGS:
                    nc.tensor.matmul(
                        acc[:, off : off + w],
                        identh[:, sgn, :],
                        hs[:, off : off + w],
                        start=(m == 0),
                        stop=(m == nm - 1),
                        skip_group_check=True,
                    )
                if interleave is not None and m < len(interleave):
                    interleave[m]()
            return acc

        def emit_horner(tB_ap, MB_ap):
            """B-region: monomial Horner for P(t) - c0 on DVE (stock ops)."""
            g = hpool.tile([P, BW], f32, tag="h")
            nc.vector.tensor_scalar_mul(out=g[:], in0=tB_ap, scalar1=pc[DEG])
            gap = g[:]
            for j in range(DEG - 1, 0, -1):
                if j == 1:
                    dst_ap = MB_ap
                else:
                    dst = hpool.tile([P, BW], f32, tag="h")
                    dst_ap = dst[:]
                nc.vector.scalar_tensor_tensor(
                    out=dst_ap,
                    in0=gap,
                    scalar=pc[j],
                    in1=tB_ap,
                    op0=OP.add,
                    op1=OP.mult,
                )
                gap = dst_ap

        def emit_merge(b, acc, Mt, split=False):
            if split:
                # halves on ACT and DVE in parallel (shorter serial chain)
                nc.scalar.copy(Mt[:, 0:ASEGS[0][1]], acc[:, 0 : ASEGS[0][1]])
                nc.vector.tensor_copy(
                    Mt[:, ASEGS[0][1] : ASPLIT], acc[:, ASEGS[0][1] : ASPLIT]
                )
            else:
                nc.scalar.copy(Mt[:, 0:ASPLIT], acc[:])

        def emit_transposes(b, Mt):
            # all 6 transposed blocks into ONE PSUM bank (ordered by J)
            tp = pstr.tile([P, NPAIR, P], bf16, tag="tr")
            for k, (I, J) in enumerate(PAIRS_BYJ):
                off = OFFS[I] + (J - I) * P
                nc.tensor.transpose(tp[:, k, :], Mt[:, off : off + P], identb)
            at = atp.tile([P, NPAIR, P], bf16, tag="at")
            nc.vector.tensor_copy(at[:], tp[:])
            return at

        def emit_contract(b, Mt, at):
            poutQ = psout.tile([DIM, N], f32, tag="q")
            poutP = psout.tile([DIM, N], f32, tag="p")
            NTOT = 2 * NCHUNK - 1  # merged contribution groups per tile
            ntouch = {id(poutQ): 0, id(poutP): 0}

            def contrib(out_cols, stat_chunk, stat_lo, mov_ap, tile_):
                k = ntouch[id(tile_)]
                ntouch[id(tile_)] = k + 1
                nc.tensor.matmul(
                    tile_[:, out_cols],
                    statx6_v(b, stat_chunk, stat_lo, stat_lo + DIM),
                    mov_ap,
                    start=(k == 0),
                    stop=(k == NTOT - 1 and tile_ is poutP),
                    skip_group_check=True,
                )

            # direct (incl. diagonal): stationary chunk I vs its whole strip
            for I in range(NCHUNK):
                mv = Mt[:, OFFS[I] : OFFS[I] + WIDTHS[I]]
                contrib(slice(P * I, N), I, 0, mv, poutQ)
                contrib(slice(P * I, N), I, DIM, mv, poutP)
            # reflected: stationary chunk J vs the J-group of transposed blocks
            for J in range(1, NCHUNK):
                g0 = J * (J - 1) // 2
                mv = at[:, g0 : g0 + J, :]
                contrib(slice(0, P * J), J, 0, mv, poutQ)
                contrib(slice(0, P * J), J, DIM, mv, poutP)
            # mu fold into Q rows: Q' = Q - mu - c0'  (muAb = -c' replicated)
            nc.tensor.matmul(
                poutQ[:, :],
                muAb,
                hmu_all[:, b, :],
                start=False,
                stop=True,
                skip_group_check=True,
            )
            return poutQ, poutP

        def emit_finalize(b, pq):
            poutQ, poutP = pq
            # out = (P' + c0*X_c) - x*(Q' + c0*N)
            o1 = finp.tile([DIM, N], f32, tag="o1")
            nc.vector.scalar_tensor_tensor(
                out=o1[:],
                in0=poutQ[:],
                scalar=c0 * float(N),
                in1=xb_sb[:, b, :],
                op0=OP.add,
                op1=OP.mult,
            )
            outrow = orp.tile([DIM, N], f32, tag="or")
            nc.vector.scalar_tensor_tensor(
                out=outrow[:],
                in0=poutP[:],
                scalar=c0x[:, b : b + 1],
                in1=o1[:],
                op0=OP.add,
                op1=OP.subtract,
            )
            nc.scalar.dma_start(out=out_d[b], in_=outrow[:])

        # ---- schedule ----
        tps0 = psd2.tile([P, NPACK], f32, tag="t0")
        for I in range(NCHUNK):
            emit_d2_chunk(0, I, tps0)
        tA0 = emit_tA(0, tps0)
        tBall = tbp.tile([P, 2 * (NPACK - ASPLIT)], f32, tag="tball")
        emit_tB(0, tps0, tBall)
        tps1 = psd2.tile([P, NPACK], f32, tag="t0")
        # interleave b1's d^2 chunks into b0's first accumulate gaps
        # (two per slot: PE stays hot, tB1 unblocks early)
        inter = [
            lambda: (emit_d2_chunk(1, 0, tps1), emit_d2_chunk(1, 1, tps1)),
            lambda: (emit_d2_chunk(1, 2, tps1), emit_d2_chunk(1, 3, tps1)),
        ]
        acc0 = emit_expacc(0, tA0[:], interleave=inter)
        Mt0 = mpool.tile([P, NPACK], bf16, tag="m0")
        emit_horner(tBall[:, 0:BW], Mt0[:, ASPLIT:NPACK])
        emit_tB(1, tps1, tBall)
        emit_merge(0, acc0, Mt0)
        acc1 = emit_expacc(1, tps1[:, 0:ASPLIT])
        # p-state filler: keep the PE busy between acc1's tail and the b1
        # transposes (dead writes into the dead d^2 tile)
        for _ in range(4):
            nc.tensor.matmul(
                tps1[:, 0:P],
                identh[:, 0, :],
                identh[:, 0, :],
                start=True,
                stop=True,
                skip_group_check=True,
            )
        at0 = emit_transposes(0, Mt0)
        Mt1 = mpool.tile([P, NPACK], bf16, tag="m1")
        emit_horner(tBall[:, BW : 2 * BW], Mt1[:, ASPLIT:NPACK])
        pq0 = emit_contract(0, Mt0, at0)
        emit_merge(1, acc1, Mt1)
        at1 = emit_transposes(1, Mt1)
        emit_finalize(0, pq0)
        pq1 = emit_contract(1, Mt1, at1)
        emit_finalize(1, pq1)

    _spread_sync_waits(nc)
    return nc


def _ensure_ntff_hook():
    """bass_utils' axon trace path imports antenv.axon_hooks, which the image's
    antenv package lacks. Register an equivalent module backed by the boot
    package's ctypes NTFF hook so trace=True works; degrade silently if the
    pieces are missing (tracing is optional)."""
    import os
    import types

    try:
        import antenv.axon_hooks  # noqa: F401

        return
    except ImportError:
        pass
    try:
        import antenv
    except ImportError:
        return
    mod = types.ModuleType("antenv.axon_hooks")
    box = {"h": None}
    mod.set_axon_ntff_profile_hook = lambda h: box.__setitem__("h", h)
    mod.get_axon_ntff_profile_hook = lambda: box["h"]
    sys.modules["antenv.axon_hooks"] = mod
    antenv.axon_hooks = mod
    try:
        from trn_agent_boot.trn_boot import _ntff_profile_via_ctypes

        so = "/opt/axon/libaxon_pjrt.so"
        if os.path.exists(so):
            hook = _ntff_profile_via_ctypes(so)
            if hook is not None:
                mod.set_axon_ntff_profile_hook(hook)
    except Exception:
        pass


def kernel(x, eta_w1, eta_b1, eta_w2, eta_b2, mu_w1, mu_b1, mu_w2, mu_b2):
    global LAST_RESULT
    _ensure_ntff_hook()
    import ml_dtypes
    from concourse.bass_utils import run_bass_kernel_spmd

    f32 = np.float32
    f16 = np.float16
    bf = ml_dtypes.bfloat16
    x = np.ascontiguousarray(np.asarray(x, dtype=f32))
    eta_w1 = np.asarray(eta_w1, f32)
    eta_b1 = np.asarray(eta_b1, f32)
    eta_w2 = np.asarray(eta_w2, f32)
    eta_b2 = np.asarray(eta_b2, f32)
    mu_w1 = np.asarray(mu_w1, f32)
    mu_b1 = np.asarray(mu_b1, f32)
    mu_w2 = np.asarray(mu_w2, f32)
    mu_b2 = np.asarray(mu_b2, f32)

    n2_all = (x.astype(np.float64) ** 2).sum(-1)  # [B, N]
    s, pc, eta_e0, eta_e1 = _fit_surrogates(x, eta_w1, eta_b1, eta_w2, eta_b2)
    mu_g, mu_c = _fit_mu_exp(n2_all, mu_w1, mu_b1, mu_w2, mu_b2)
    c0 = float(pc[0])

    nc = _build_program(pc, eta_e0, eta_e1)

    DR = DIM + 2
    identh = np.empty((P, 2, P), f16)
    identh[:, 0, :] = np.eye(P, dtype=f32)
    identh[:, 1, :] = -np.eye(P, dtype=f32)
    ea_bias0 = eta_e0[0] + np.log(np.abs(eta_e0[1]))
    ea_bias1 = eta_e1[0] + np.log(np.abs(eta_e1[1]))

    BPK = BPC * NCHUNK * 2 * DIM + P + DIM

    in_maps = []
    for core in range(NCORES):
        xc = x[core * BPC : (core + 1) * BPC]  # [BPC, N, DIM]
        xTc = xc.transpose(0, 2, 1)  # [BPC, DIM, N]
        n2 = n2_all[core * BPC : (core + 1) * BPC].astype(f32)  # [BPC, N]
        dfirst = np.empty((DR, P + N), f32)
        dpack = np.empty((DR, 7 * P + N), f32)
        dfirst[0:DIM, P : P + N] = xTc[0]
        dfirst[DIM, P : P + N] = n2[0]
        dfirst[DIM + 1, P : P + N] = 1.0
        dpack[0:DIM, 7 * P :] = xTc[1]
        dpack[DIM, 7 * P :] = n2[1]
        dpack[DIM + 1, 7 * P :] = 1.0
        bpack = np.zeros((P, BPK), f32)
        for bb in range(BPC):
            for I in range(NCHUNK):
                scol = np.empty((DR, P), f32)
                scol[0:DIM] = -2.0 * s * xTc[bb, :, I * P : (I + 1) * P]
                scol[DIM] = s
                scol[DIM + 1] = s * n2[bb, I * P : (I + 1) * P] - 1.0
                if bb == 0 and I == 0:
                    dfirst[:, 0:P] = scol
                else:
                    soff = (bb * NCHUNK + I - 1) * P
                    dpack[:, soff : soff + P] = scol
                boff = (bb * NCHUNK + I) * 2 * DIM
                bpack[:, boff : boff + DIM] = 1.0
                bpack[:, boff + DIM : boff + 2 * DIM] = xc[bb, I * P : (I + 1) * P, :]
        bpack[:, BPC * NCHUNK * 2 * DIM : BPC * NCHUNK * 2 * DIM + P] = np.eye(P)
        bpack[0:MU, BPK - DIM : BPK] = np.repeat(-mu_c[:, None], DIM, axis=1)
        fpack = np.zeros((P, FPK), f32)
        fpack[:, 0:META] = ea_bias0[None, :].astype(f32)
        fpack[:, META : META + META1] = ea_bias1[None, :].astype(f32)
        fpack[0:MU, META + META1 + 1] = -mu_g
        fpack[0:DIM, META + META1 + 2 : META + META1 + 2 + BPC] = c0 * xc.sum(axis=1).T
        unrep = np.broadcast_to(n2[None, :, :], (MU, BPC, N))
        in_maps.append(
            {
                "dfirst": dfirst.astype(f16),
                "dpack": dpack.astype(f16),
                "identh": identh,
                "bpack": bpack.astype(bf),
                "xb": np.ascontiguousarray(xTc.transpose(1, 0, 2)).astype(bf),
                "unrep": np.ascontiguousarray(unrep).astype(f16),
                "fpack": fpack,
            }
        )

    res = run_bass_kernel_spmd(nc, in_maps, core_ids=list(range(NCORES)))
    LAST_RESULT = res
    out = np.concatenate([r["out"] for r in res.results], axis=0)  # [B, DIM, N]
    return np.ascontiguousarray(out.transpose(0, 2, 1)).astype(np.float32)
